# revision 15
# baseline (speedup 1.0000x reference)
"""Multi-head attention (whisper-style, returns (out, qk)) on 8 Trainium2 cores.

Sharding: core c -> (batch b = c//2, head-group hg = c%2). Each core computes
8 heads (512 features) of one batch: QKV projections, causal attention scores
(returned as qk), softmax, attention-weighted V, and a partial output
projection. Host sums the two head-group partials per batch and adds bo.

All matmuls run in float32r (TF32-like fast path). Causal mask applied on
device via affine_select (-inf fill) for the stored qk; softmax path uses a
transposed score layout with triangular skipping.
"""

import sys

sys.path.insert(0, "/opt/trn_rl_repo")

import numpy as np

import concourse.bass as bass  # noqa: F401  (import registers AP machinery)
from concourse import bacc, bass_utils, mybir
import concourse.tile as tile

B, T, D, H = 4, 1500, 1024, 16
DH = D // H              # 64
NCORES = 8
HPC = H // 2             # 8 heads per core
FPC = HPC * DH           # 512 features per core
NT = (T + 127) // 128    # 12 partition tiles over T (last has 92 rows)
# Column chunk boundaries over T, aligned to the 512-float fp32 PSUM bank
# (a matmul output must not cross a bank boundary).
CS = [0, 512, 1024, T]
NJ = len(CS) - 1
VW = DH + 1              # v columns per head incl. ones column (65)

f32 = mybir.dt.float32
f32r = mybir.dt.float32r
EXP = mybir.ActivationFunctionType.Exp
IDENT = mybir.ActivationFunctionType.Identity
GE = mybir.AluOpType.is_ge

_cached_nc = None


def _trows(i):
    return min(128, T - 128 * i)


def _build():
    nc = bacc.Bacc("TRN2", target_bir_lowering=False, debug=False)

    xT = nc.dram_tensor("xT", [D + 1, T], f32r, kind="ExternalInput").ap()
    wqT = nc.dram_tensor("wqT", [D, FPC], f32r, kind="ExternalInput").ap()
    wkT = nc.dram_tensor("wkT", [D, FPC], f32r, kind="ExternalInput").ap()
    wvT = nc.dram_tensor("wvT", [D + 1, HPC * VW], f32r, kind="ExternalInput").ap()
    woT = nc.dram_tensor("woT", [FPC, D], f32r, kind="ExternalInput").ap()
    bqv = nc.dram_tensor("bq", [FPC, 1], f32, kind="ExternalInput").ap()
    qk_out = nc.dram_tensor("qk_out", [HPC, T, T], f32, kind="ExternalOutput").ap()
    outT = nc.dram_tensor("outT", [D, T], f32, kind="ExternalOutput").ap()

    with tile.TileContext(nc) as tc:
        # ---------------- persistent SBUF ----------------
        with tc.tile_pool(name="perm", bufs=1) as perm:
            qT_sb = [perm.tile([128, T], f32r, name=f"qT{m}") for m in range(4)]
            kT_sb = [perm.tile([128, T], f32r, name=f"kT{m}") for m in range(4)]
            v_sb = [perm.tile([128, HPC * VW], f32r, name=f"v{i}") for i in range(NT)]
            oT_sb = [perm.tile([128, T], f32r, name=f"oT{m}") for m in range(4)]

            # ---------------- phase 1: QKV projections ----------------
            with tc.tile_pool(name="xw", bufs=1) as xw:
                xT_sb = [xw.tile([128, T], f32r, name=f"x{k}") for k in range(8)]
                for k in range(8):
                    nc.sync.dma_start(xT_sb[k][:], xT[128 * k : 128 * (k + 1), :])
                xT1_sb = xw.tile([1, T], f32r, name="x_ones")
                nc.sync.dma_start(xT1_sb[:], xT[D : D + 1, :])

                # q^T = (Wq_p*s) @ x^T + bq*s   (bias via ACT on evacuation)
                # k^T = (Wk_p*s) @ x^T
                with tc.tile_pool(name="wq", bufs=1) as wq, \
                     tc.tile_pool(name="qkps", bufs=2, space="PSUM") as qkps:
                    wq_sb = [wq.tile([128, FPC], f32r, name=f"wq{k}") for k in range(8)]
                    bq_sb = [wq.tile([128, 1], f32, name=f"bq{m}") for m in range(4)]
                    for k in range(8):
                        nc.sync.dma_start(wq_sb[k][:], wqT[128 * k : 128 * (k + 1), :])
                    for m in range(4):
                        nc.sync.dma_start(bq_sb[m][:], bqv[128 * m : 128 * (m + 1), :])
                    for m in range(4):
                        qp = qkps.tile([128, T], f32, tag="qkp")
                        for j in range(NJ):
                            for k in range(8):
                                nc.tensor.matmul(
                                    qp[:, CS[j] : CS[j + 1]],
                                    wq_sb[k][:, 128 * m : 128 * (m + 1)],
                                    xT_sb[k][:, CS[j] : CS[j + 1]],
                                    start=(k == 0), stop=(k == 7),
                                )
                        nc.scalar.activation(qT_sb[m][:], qp[:], IDENT, bias=bq_sb[m][:])

                    wk_sb = [wq.tile([128, FPC], f32r, name=f"wk{k}") for k in range(8)]
                    for k in range(8):
                        nc.sync.dma_start(wk_sb[k][:], wkT[128 * k : 128 * (k + 1), :])
                    for m in range(4):
                        kp = qkps.tile([128, T], f32, tag="qkp")
                        for j in range(NJ):
                            for k in range(8):
                                nc.tensor.matmul(
                                    kp[:, CS[j] : CS[j + 1]],
                                    wk_sb[k][:, 128 * m : 128 * (m + 1)],
                                    xT_sb[k][:, CS[j] : CS[j + 1]],
                                    start=(k == 0), stop=(k == 7),
                                )
                        nc.vector.tensor_copy(kT_sb[m][:], kp[:])

                # v = x @ Wv_p^T + bv  (bias via ones-row of xT / bv-row of wvT)
                # wvT is pre-interleaved on host: per head [wv_h (64) | e] where
                # the extra column is 0 except 1.0 in the bias row -> v_sb gets
                # [v_h | 1] * 8 directly (520 cols, 2 psum banks).
                NV = HPC * VW
                with tc.tile_pool(name="wv", bufs=1) as wv, \
                     tc.tile_pool(name="vps", bufs=2, space="PSUM") as vps:
                    wv_sb = [wv.tile([128, NV], f32r, name=f"wv{k}") for k in range(8)]
                    for k in range(8):
                        nc.sync.dma_start(wv_sb[k][:], wvT[128 * k : 128 * (k + 1), :])
                    wv1_sb = wv.tile([1, NV], f32r, name="wv_bias")
                    nc.sync.dma_start(wv1_sb[:], wvT[D : D + 1, :])
                    for i in range(NT):
                        rw = _trows(i)
                        vp = vps.tile([128, NV], f32, tag="vp")
                        for ca, cb in ((0, 512), (512, NV)):
                            for k in range(8):
                                nc.tensor.matmul(
                                    vp[0:rw, ca:cb],
                                    xT_sb[k][:, 128 * i : 128 * i + rw],
                                    wv_sb[k][:, ca:cb],
                                    start=(k == 0), stop=False,
                                )
                            nc.tensor.matmul(
                                vp[0:rw, ca:cb],
                                xT1_sb[:, 128 * i : 128 * i + rw],
                                wv1_sb[:, ca:cb],
                                start=False, stop=True,
                            )
                        nc.vector.tensor_copy(v_sb[i][0:rw, :], vp[0:rw, :])

            # ---------------- phase 2: attention per head ----------------
            with tc.tile_pool(name="sstage", bufs=4) as sstage, \
                 tc.tile_pool(name="et", bufs=10) as etp, \
                 tc.tile_pool(name="nrm", bufs=3) as nrm, \
                 tc.tile_pool(name="sps", bufs=2, space="PSUM") as sps, \
                 tc.tile_pool(name="stps", bufs=3, space="PSUM") as stps, \
                 tc.tile_pool(name="pops", bufs=3, space="PSUM") as pops:
                for h in range(HPC):
                    ht, fo = divmod(h, 2)
                    fo *= DH
                    qh = qT_sb[ht][fo : fo + DH, :]
                    kh = kT_sb[ht][fo : fo + DH, :]

                    # --- scores S[tq, tk] for the qk output (valid prefix) ---
                    for i in range(NT):
                        rw = _trows(i)
                        vend = min(128 * i + 128, T)     # cols [0, vend) written
                        st = sstage.tile([128, T], f32, tag="sstage")
                        for j in range(NJ):
                            if CS[j] >= vend:
                                break
                            ce = min(CS[j + 1], vend)
                            wj = CS[j + 1] - CS[j]
                            sp = sps.tile([128, 512], f32, tag="sp")
                            nc.tensor.matmul(
                                sp[0:rw, 0:wj],
                                qh[:, 128 * i : 128 * i + rw],
                                kh[:, CS[j] : CS[j + 1]],
                                start=True, stop=True,
                            )
                            nc.vector.tensor_copy(
                                st[0:rw, CS[j] : ce], sp[0:rw, 0 : ce - CS[j]]
                            )
                        # causal -inf fill on the diagonal window [128i, vend)
                        nc.gpsimd.affine_select(
                            st[0:rw, 128 * i : vend],
                            st[0:rw, 128 * i : vend],
                            pattern=[[-1, vend - 128 * i]],
                            compare_op=GE, fill=float("-inf"),
                            base=0, channel_multiplier=1,
                        )
                        nc.sync.dma_start(
                            qk_out[h, 128 * i : 128 * i + rw, 0:vend], st[0:rw, 0:vend]
                        )

                    # --- transposed scores -> exp -> E^T -> po += v_aug.T @ E^T ---
                    for j in range(NJ):
                        wj = CS[j + 1] - CS[j]
                        po = pops.tile([VW, 512], f32, tag="po")
                        ms = [m for m in range(NT) if 128 * m < CS[j + 1]]
                        for n, m in enumerate(ms):
                            rw = _trows(m)
                            c0 = 128 * m                 # valid tq suffix start
                            stp = stps.tile([128, 512], f32, tag="stp")
                            nc.tensor.matmul(
                                stp[0:rw, 0:wj],
                                kh[:, c0 : c0 + rw],
                                qh[:, CS[j] : CS[j + 1]],
                                start=True, stop=True,
                            )
                            et = etp.tile([128, 512], f32r, tag="et")
                            cs = max(c0 - CS[j], 0)      # in-tile valid start
                            nc.scalar.activation(
                                et[0:rw, cs:wj], stp[0:rw, cs:wj], EXP
                            )
                            # zero everything strictly below the diagonal
                            # (covers the unwritten [0, cs) prefix too:
                            # keep where tq - tk >= 0, else fill 0)
                            me = min(c0 + 128, CS[j + 1]) - CS[j]
                            if c0 + 128 > CS[j] and me > 0:
                                nc.gpsimd.affine_select(
                                    et[0:rw, 0:me],
                                    et[0:rw, 0:me],
                                    pattern=[[1, me]],
                                    compare_op=GE, fill=0.0,
                                    base=CS[j] - c0, channel_multiplier=-1,
                                )
                            nc.tensor.matmul(
                                po[:, 0:wj],
                                v_sb[m][0:rw, VW * h : VW * (h + 1)],
                                et[0:rw, 0:wj],
                                start=(n == 0), stop=(n == len(ms) - 1),
                            )

                        # --- normalize: out_h^T[:, j] = po[0:64] * (1/po[64]) ---
                        dsb = nrm.tile([1, 512], f32, tag="dsb")
                        nc.vector.tensor_copy(dsb[0:1, 0:wj], po[DH : DH + 1, 0:wj])
                        rsb = nrm.tile([1, 512], f32, tag="rsb")
                        nc.vector.reciprocal(rsb[0:1, 0:wj], dsb[0:1, 0:wj])
                        rbc = nrm.tile([DH, 512], f32, tag="rbc")
                        nc.gpsimd.partition_broadcast(rbc[:, 0:wj], rsb[0:1, 0:wj])
                        nc.vector.tensor_mul(
                            oT_sb[ht][fo : fo + DH, CS[j] : CS[j + 1]],
                            po[0:DH, 0:wj], rbc[:, 0:wj],
                        )

            # ---------------- phase 3: output projection ----------------
            with tc.tile_pool(name="wo", bufs=1) as wo, \
                 tc.tile_pool(name="ostage", bufs=2) as ostage, \
                 tc.tile_pool(name="ops", bufs=2, space="PSUM") as ops:
                wo_sb = [wo.tile([128, D], f32r, name=f"wo{k}") for k in range(4)]
                for k in range(4):
                    nc.sync.dma_start(wo_sb[k][:], woT[128 * k : 128 * (k + 1), :])
                for n in range(8):
                    pp = ops.tile([128, T], f32, tag="pp")
                    for j in range(NJ):
                        for k in range(4):
                            nc.tensor.matmul(
                                pp[:, CS[j] : CS[j + 1]],
                                wo_sb[k][:, 128 * n : 128 * (n + 1)],
                                oT_sb[k][:, CS[j] : CS[j + 1]],
                                start=(k == 0), stop=(k == 3),
                            )
                    ot = ostage.tile([128, T], f32, tag="ot")
                    nc.scalar.activation(ot[:], pp[:], IDENT)
                    nc.sync.dma_start(outT[128 * n : 128 * (n + 1), :], ot[:])

    nc.compile()
    return nc


def _get_nc():
    global _cached_nc
    if _cached_nc is None:
        _cached_nc = _build()
    return _cached_nc


def kernel(x, mask, Wq, bq, Wk, Wv, bv, Wo, bo, _run_kwargs=None):
    x = np.asarray(x, dtype=np.float32)
    Wq = np.asarray(Wq, dtype=np.float32)
    bq = np.asarray(bq, dtype=np.float32)
    Wk = np.asarray(Wk, dtype=np.float32)
    Wv = np.asarray(Wv, dtype=np.float32)
    bv = np.asarray(bv, dtype=np.float32)
    Wo = np.asarray(Wo, dtype=np.float32)
    bo = np.asarray(bo, dtype=np.float32)

    nc = _get_nc()
    s = float(DH) ** -0.25

    in_maps = []
    for c in range(NCORES):
        b, hg = divmod(c, 2)
        sl = slice(hg * FPC, (hg + 1) * FPC)
        xT_aug = np.empty((D + 1, T), np.float32)
        xT_aug[:D] = x[b].T
        xT_aug[D] = 1.0
        wvT_aug = np.zeros((D + 1, HPC * VW), np.float32)
        wvt = Wv[sl].T
        for hl in range(HPC):
            wvT_aug[:D, hl * VW : hl * VW + DH] = wvt[:, hl * DH : (hl + 1) * DH]
            wvT_aug[D, hl * VW : hl * VW + DH] = bv[sl][hl * DH : (hl + 1) * DH]
            wvT_aug[D, hl * VW + DH] = 1.0
        in_maps.append({
            "xT": xT_aug,
            "wqT": np.ascontiguousarray((Wq[sl] * s).T),
            "wkT": np.ascontiguousarray((Wk[sl] * s).T),
            "wvT": wvT_aug,
            "woT": np.ascontiguousarray(Wo[:, sl].T),
            "bq": (bq[sl] * s).reshape(FPC, 1).astype(np.float32),
        })

    res = bass_utils.run_bass_kernel_spmd(
        nc, in_maps, core_ids=list(range(NCORES)), **(_run_kwargs or {})
    )

    out = np.empty((B, T, D), np.float32)
    qk = np.empty((B, H, T, T), np.float32)
    for b in range(B):
        r0 = res.results[2 * b]
        r1 = res.results[2 * b + 1]
        out[b] = r0["outT"].T + r1["outT"].T + bo
        for hg, r in ((0, r0), (1, r1)):
            for hl in range(HPC):
                h = hg * HPC + hl
                dst = qk[b, h]
                src = r["qk_out"][hl]
                for i in range(NT):
                    ra, rb = 128 * i, 128 * i + _trows(i)
                    vend = min(128 * i + 128, T)
                    dst[ra:rb, :vend] = src[ra:rb, :vend]
                    dst[ra:rb, vend:] = -np.inf
    if _run_kwargs is not None:
        return (out, qk), res
    return out, qk


# revision 19
# speedup vs baseline: 1.2586x; 1.2586x over previous
"""Multi-head attention (whisper-style, returns (out, qk)) on 8 Trainium2 cores.

Sharding: core c -> (batch b = c//2, head-group hg = c%2). Each core computes
8 heads (512 features) of one batch: QKV projections, causal attention scores
(returned as qk), softmax, attention-weighted V, and a partial output
projection. Host sums the two head-group partials per batch and adds bo.

All matmuls run in float32r (TF32-like fast path). Heads are processed in
pairs: the two heads of a qT/kT tile live at partitions 0-63 / 64-127, and
their K=64 score matmuls are packed into the PE array concurrently via
tile_position row groups, writing adjacent PSUM banks. exp / copies / causal
masking / DMA are fused across the pair with 3D access patterns.
"""

import sys

sys.path.insert(0, "/opt/trn_rl_repo")

import numpy as np

import concourse.bass as bass  # noqa: F401  (import registers AP machinery)
from concourse import bacc, bass_utils, mybir
import concourse.tile as tile

B, T, D, H = 4, 1500, 1024, 16
DH = D // H              # 64
NCORES = 8
HPC = H // 2             # 8 heads per core
FPC = HPC * DH           # 512 features per core
NT = (T + 127) // 128    # 12 partition tiles over T (last has 92 rows)
# Column chunk boundaries over T, aligned to the 512-float fp32 PSUM bank
# (a matmul output must not cross a bank boundary).
CS = [0, 512, 1024, T]
NJ = len(CS) - 1
VW = DH + 1              # v columns per head incl. ones column (65)
NV = HPC * VW            # 520

f32 = mybir.dt.float32
f32r = mybir.dt.float32r
EXP = mybir.ActivationFunctionType.Exp
IDENT = mybir.ActivationFunctionType.Identity
GE = mybir.AluOpType.is_ge

_cached_nc = None


def _trows(i):
    return min(128, T - 128 * i)


def _build():
    nc = bacc.Bacc("TRN2", target_bir_lowering=False, debug=False)

    xT = nc.dram_tensor("xT", [D, T], f32r, kind="ExternalInput").ap()
    wqT = nc.dram_tensor("wqT", [D, FPC], f32r, kind="ExternalInput").ap()
    wkT = nc.dram_tensor("wkT", [D, FPC], f32r, kind="ExternalInput").ap()
    wvT = nc.dram_tensor("wvT", [D, FPC], f32r, kind="ExternalInput").ap()
    woT = nc.dram_tensor("woT", [FPC, D], f32r, kind="ExternalInput").ap()
    bqv = nc.dram_tensor("bq", [FPC, 1], f32, kind="ExternalInput").ap()
    bvv = nc.dram_tensor("bv", [1, FPC], f32, kind="ExternalInput").ap()
    vones = nc.dram_tensor("vones", [128, HPC, 1], f32r, kind="ExternalInput").ap()
    qk_out = nc.dram_tensor("qk_out", [HPC, T, T], f32, kind="ExternalOutput").ap()
    outT = nc.dram_tensor("outT", [D, T], f32, kind="ExternalOutput").ap()

    with tile.TileContext(nc) as tc:
        # ---------------- persistent SBUF ----------------
        with tc.tile_pool(name="perm", bufs=1) as perm:
            qT_sb = [perm.tile([128, T], f32r, name=f"qT{m}") for m in range(4)]
            kT_sb = [perm.tile([128, T], f32r, name=f"kT{m}") for m in range(4)]
            v_sb = [perm.tile([128, NV], f32r, name=f"v{i}") for i in range(NT)]
            oT_sb = [perm.tile([128, T], f32r, name=f"oT{m}") for m in range(4)]

            # ---------------- phase 1: QKV projections ----------------
            with tc.tile_pool(name="xw", bufs=1) as xw:
                xT_sb = [xw.tile([128, T], f32r, name=f"x{k}") for k in range(8)]
                for k in range(8):
                    nc.sync.dma_start(xT_sb[k][:], xT[128 * k : 128 * (k + 1), :])

                # q^T = (Wq_p*s) @ x^T (+ bq*s via ACT bias on evacuation)
                # k^T = (Wk_p*s) @ x^T
                with tc.tile_pool(name="wq", bufs=1) as wq, \
                     tc.tile_pool(name="qkps", bufs=2, space="PSUM") as qkps:
                    wq_sb = [wq.tile([128, FPC], f32r, name=f"wq{k}") for k in range(8)]
                    bq_sb = [wq.tile([128, 1], f32, name=f"bq{m}") for m in range(4)]
                    for k in range(8):
                        nc.sync.dma_start(wq_sb[k][:], wqT[128 * k : 128 * (k + 1), :])
                    for m in range(4):
                        nc.sync.dma_start(bq_sb[m][:], bqv[128 * m : 128 * (m + 1), :])
                    for m in range(4):
                        qp = qkps.tile([128, T], f32, tag="qkp")
                        for k in range(8):
                            for j in range(NJ):
                                nc.tensor.matmul(
                                    qp[:, CS[j] : CS[j + 1]],
                                    wq_sb[k][:, 128 * m : 128 * (m + 1)],
                                    xT_sb[k][:, CS[j] : CS[j + 1]],
                                    start=(k == 0), stop=(k == 7),
                                )
                        nc.scalar.activation(qT_sb[m][:], qp[:], IDENT, bias=bq_sb[m][:])

                    wk_sb = [wq.tile([128, FPC], f32r, name=f"wk{k}") for k in range(8)]
                    for k in range(8):
                        nc.sync.dma_start(wk_sb[k][:], wkT[128 * k : 128 * (k + 1), :])
                    for m in range(4):
                        kp = qkps.tile([128, T], f32, tag="qkp")
                        for k in range(8):
                            for j in range(NJ):
                                nc.tensor.matmul(
                                    kp[:, CS[j] : CS[j + 1]],
                                    wk_sb[k][:, 128 * m : 128 * (m + 1)],
                                    xT_sb[k][:, CS[j] : CS[j + 1]],
                                    start=(k == 0), stop=(k == 7),
                                )
                        nc.vector.tensor_copy(kT_sb[m][:], kp[:])

                # v = x @ Wv_p^T + bv; stored interleaved [v_h (64) | 1] * 8.
                # bv is added on evacuation (partition-broadcast once); the
                # ones columns come from a tiny constant DMA.
                with tc.tile_pool(name="wv", bufs=1) as wv, \
                     tc.tile_pool(name="vps", bufs=2, space="PSUM") as vps:
                    wv_sb = [wv.tile([128, FPC], f32r, name=f"wv{k}") for k in range(8)]
                    for k in range(8):
                        nc.sync.dma_start(wv_sb[k][:], wvT[128 * k : 128 * (k + 1), :])
                    bv_row = wv.tile([1, FPC], f32, name="bv_row")
                    nc.sync.dma_start(bv_row[:], bvv[:])
                    bv_bc = wv.tile([128, FPC], f32, name="bv_bc")
                    nc.gpsimd.partition_broadcast(bv_bc[:], bv_row[:])
                    for i in range(NT):
                        rw = _trows(i)
                        nc.sync.dma_start(
                            v_sb[i][0:rw].rearrange("p (h c) -> p h c", c=VW)[:, :, DH : DH + 1],
                            vones[0:rw],
                        )
                        vp = vps.tile([128, FPC], f32, tag="vp")
                        for k in range(8):
                            nc.tensor.matmul(
                                vp[0:rw, :],
                                xT_sb[k][:, 128 * i : 128 * i + rw],
                                wv_sb[k][:],
                                start=(k == 0), stop=(k == 7),
                            )
                        nc.vector.tensor_add(
                            v_sb[i][0:rw].rearrange("p (h c) -> p h c", c=VW)[:, :, 0:DH],
                            vp[0:rw].rearrange("p (h c) -> p h c", c=DH),
                            bv_bc[0:rw].rearrange("p (h c) -> p h c", c=DH),
                        )

            # ---------------- phase 2: attention, head pairs ----------------
            # pair t: head A = 2t (partitions 0:64), head B = 2t+1 (64:128)
            with tc.tile_pool(name="sstage", bufs=3) as sstage, \
                 tc.tile_pool(name="et", bufs=6) as etp, \
                 tc.tile_pool(name="nrm", bufs=3) as nrm, \
                 tc.tile_pool(name="sps", bufs=1, space="PSUM") as sps, \
                 tc.tile_pool(name="stps", bufs=2, space="PSUM") as stps, \
                 tc.tile_pool(name="pops", bufs=1, space="PSUM") as pops:
                for t in range(4):
                    hA = 2 * t
                    qt, kt = qT_sb[t], kT_sb[t]

                    # --- scores S[tq, tk] for the qk output (valid prefix) ---
                    # paired: bank0 = head A, bank1 = head B; st cols
                    # [0,T) = A, [T,2T) = B
                    for i in range(NT):
                        rw = _trows(i)
                        vend = min(128 * i + 128, T)
                        st = sstage.tile([128, 2 * T], f32, tag="sstage")
                        for j in range(NJ):
                            if CS[j] >= vend:
                                break
                            ce = min(CS[j + 1], vend)
                            w = ce - CS[j]
                            sp = sps.tile([128, 1024], f32, tag="sp")
                            nc.tensor.matmul(
                                sp[0:rw, 0 : CS[j + 1] - CS[j]],
                                qt[0:64, 128 * i : 128 * i + rw],
                                kt[0:64, CS[j] : CS[j + 1]],
                                start=True, stop=True, tile_position=(0, 0),
                            )
                            nc.tensor.matmul(
                                sp[0:rw, 512 : 512 + CS[j + 1] - CS[j]],
                                qt[64:128, 128 * i : 128 * i + rw],
                                kt[64:128, CS[j] : CS[j + 1]],
                                start=True, stop=True, tile_position=(64, 0),
                            )
                            # one paired copy: ACT for j==0 (bigger share on
                            # DVE hurts; balance exp-heavy ACT vs copy-heavy
                            # DVE by splitting on j)
                            src = sp[0:rw].rearrange("p (g c) -> p g c", c=512)[:, :, 0:w]
                            dst = st[0:rw].rearrange("p (g c) -> p g c", c=T)[:, :, CS[j] : ce]
                            if (i + j) % 3 == 0:
                                nc.scalar.activation(dst, src, IDENT)
                            else:
                                nc.vector.tensor_copy(dst, src)
                        # causal -inf fill on the diagonal window [128i, vend)
                        nc.gpsimd.affine_select(
                            st[0:rw].rearrange("p (g c) -> p g c", c=T)[:, :, 128 * i : vend],
                            st[0:rw].rearrange("p (g c) -> p g c", c=T)[:, :, 128 * i : vend],
                            pattern=[[0, 2], [-1, vend - 128 * i]],
                            compare_op=GE, fill=float("-inf"),
                            base=0, channel_multiplier=1,
                        )
                        nc.sync.dma_start(
                            qk_out[hA : hA + 2, 128 * i : 128 * i + rw, 0:vend]
                            .rearrange("h r c -> r h c"),
                            st[0:rw].rearrange("p (g c) -> p g c", c=T)[:, :, 0:vend],
                        )

                    # --- S^T -> exp -> E^T -> po += v_aug.T @ E^T, paired ---
                    for j in range(NJ):
                        wj = CS[j + 1] - CS[j]
                        po = pops.tile([VW, 1024], f32, tag="po")
                        ms = [m for m in range(NT) if 128 * m < CS[j + 1]]
                        for n, m in enumerate(ms):
                            rw = _trows(m)
                            c0 = 128 * m
                            stp = stps.tile([128, 1024], f32, tag="stp")
                            nc.tensor.matmul(
                                stp[0:rw, 0:wj],
                                kt[0:64, c0 : c0 + rw],
                                qt[0:64, CS[j] : CS[j + 1]],
                                start=True, stop=True, tile_position=(0, 0),
                            )
                            nc.tensor.matmul(
                                stp[0:rw, 512 : 512 + wj],
                                kt[64:128, c0 : c0 + rw],
                                qt[64:128, CS[j] : CS[j + 1]],
                                start=True, stop=True, tile_position=(64, 0),
                            )
                            et = etp.tile([128, 1024], f32r, tag="et")
                            cs = max(c0 - CS[j], 0)      # in-tile valid start
                            nc.scalar.activation(
                                et[0:rw].rearrange("p (g c) -> p g c", c=512)[:, :, cs:wj],
                                stp[0:rw].rearrange("p (g c) -> p g c", c=512)[:, :, cs:wj],
                                EXP,
                            )
                            # zero strictly-below-diagonal (and the unwritten
                            # [0, cs) prefix): keep where tq - tk >= 0
                            me = min(c0 + 128, CS[j + 1]) - CS[j]
                            if c0 + 128 > CS[j] and me > 0:
                                nc.gpsimd.affine_select(
                                    et[0:rw].rearrange("p (g c) -> p g c", c=512)[:, :, 0:me],
                                    et[0:rw].rearrange("p (g c) -> p g c", c=512)[:, :, 0:me],
                                    pattern=[[0, 2], [1, me]],
                                    compare_op=GE, fill=0.0,
                                    base=CS[j] - c0, channel_multiplier=-1,
                                )
                            nc.tensor.matmul(
                                po[:, 0:wj],
                                v_sb[m][0:rw, VW * hA : VW * hA + VW],
                                et[0:rw, 0:wj],
                                start=(n == 0), stop=(n == len(ms) - 1),
                            )
                            nc.tensor.matmul(
                                po[:, 512 : 512 + wj],
                                v_sb[m][0:rw, VW * (hA + 1) : VW * (hA + 1) + VW],
                                et[0:rw, 512 : 512 + wj],
                                start=(n == 0), stop=(n == len(ms) - 1),
                            )

                        # --- normalize: out_h^T[:, j] = po[0:64] * (1/po[64]) ---
                        rsb = nrm.tile([1, 1024], f32, tag="rsb")
                        pod = po[DH : DH + 1].rearrange("p (g c) -> p g c", c=512)[:, :, 0:wj]
                        rsbv = rsb.rearrange("p (g c) -> p g c", c=512)[:, :, 0:wj]
                        nc.vector.reciprocal(rsbv, pod)
                        rbc = nrm.tile([DH, 1024], f32, tag="rbc")
                        nc.gpsimd.partition_broadcast(rbc[:], rsb[:])
                        nc.vector.tensor_mul(
                            oT_sb[t][0:DH, CS[j] : CS[j + 1]],
                            po[0:DH, 0:wj], rbc[:, 0:wj],
                        )
                        nc.vector.tensor_mul(
                            oT_sb[t][DH:128, CS[j] : CS[j + 1]],
                            po[0:DH, 512 : 512 + wj], rbc[:, 512 : 512 + wj],
                        )

            # ---------------- phase 3: output projection ----------------
            with tc.tile_pool(name="wo", bufs=1) as wo, \
                 tc.tile_pool(name="ostage", bufs=2) as ostage, \
                 tc.tile_pool(name="ops", bufs=2, space="PSUM") as ops:
                wo_sb = [wo.tile([128, D], f32r, name=f"wo{k}") for k in range(4)]
                for k in range(4):
                    nc.sync.dma_start(wo_sb[k][:], woT[128 * k : 128 * (k + 1), :])
                for n in range(8):
                    pp = ops.tile([128, T], f32, tag="pp")
                    for k in range(4):
                        for j in range(NJ):
                            nc.tensor.matmul(
                                pp[:, CS[j] : CS[j + 1]],
                                wo_sb[k][:, 128 * n : 128 * (n + 1)],
                                oT_sb[k][:, CS[j] : CS[j + 1]],
                                start=(k == 0), stop=(k == 3),
                            )
                    ot = ostage.tile([128, T], f32, tag="ot")
                    nc.scalar.activation(ot[:], pp[:], IDENT)
                    nc.sync.dma_start(outT[128 * n : 128 * (n + 1), :], ot[:])

    nc.compile()
    return nc


def _get_nc():
    global _cached_nc
    if _cached_nc is None:
        _cached_nc = _build()
    return _cached_nc


def kernel(x, mask, Wq, bq, Wk, Wv, bv, Wo, bo, _run_kwargs=None):
    x = np.asarray(x, dtype=np.float32)
    Wq = np.asarray(Wq, dtype=np.float32)
    bq = np.asarray(bq, dtype=np.float32)
    Wk = np.asarray(Wk, dtype=np.float32)
    Wv = np.asarray(Wv, dtype=np.float32)
    bv = np.asarray(bv, dtype=np.float32)
    Wo = np.asarray(Wo, dtype=np.float32)
    bo = np.asarray(bo, dtype=np.float32)

    nc = _get_nc()
    s = float(DH) ** -0.25

    in_maps = []
    for c in range(NCORES):
        b, hg = divmod(c, 2)
        sl = slice(hg * FPC, (hg + 1) * FPC)
        in_maps.append({
            "xT": np.ascontiguousarray(x[b].T),
            "wqT": np.ascontiguousarray((Wq[sl] * s).T),
            "wkT": np.ascontiguousarray((Wk[sl] * s).T),
            "wvT": np.ascontiguousarray(Wv[sl].T),
            "woT": np.ascontiguousarray(Wo[:, sl].T),
            "bq": (bq[sl] * s).reshape(FPC, 1).astype(np.float32),
            "bv": bv[sl].reshape(1, FPC).astype(np.float32),
            "vones": np.ones((128, HPC, 1), np.float32),
        })

    res = bass_utils.run_bass_kernel_spmd(
        nc, in_maps, core_ids=list(range(NCORES)), **(_run_kwargs or {})
    )

    out = np.empty((B, T, D), np.float32)
    qk = np.empty((B, H, T, T), np.float32)
    for b in range(B):
        r0 = res.results[2 * b]
        r1 = res.results[2 * b + 1]
        out[b] = r0["outT"].T + r1["outT"].T + bo
        for hg, r in ((0, r0), (1, r1)):
            for hl in range(HPC):
                h = hg * HPC + hl
                dst = qk[b, h]
                src = r["qk_out"][hl]
                for i in range(NT):
                    ra, rb = 128 * i, 128 * i + _trows(i)
                    vend = min(128 * i + 128, T)
                    dst[ra:rb, :vend] = src[ra:rb, :vend]
                    dst[ra:rb, vend:] = -np.inf
    if _run_kwargs is not None:
        return (out, qk), res
    return out, qk


# revision 22
# speedup vs baseline: 1.3535x; 1.0754x over previous
"""Multi-head attention (whisper-style, returns (out, qk)) on 8 Trainium2 cores.

Sharding: core c -> (batch b = c//2, head-group hg = c%2). Each core computes
8 heads (512 features) of one batch: QKV projections, causal attention scores
(returned as qk), softmax, attention-weighted V, and a partial output
projection. Host sums the two head-group partials per batch and adds bo.

All matmuls run in float32r (TF32-like fast path). Heads are processed in
pairs: the two heads of a qT/kT tile live at partitions 0-63 / 64-127, and
their K=64 score matmuls are packed into the PE array concurrently via
tile_position row groups, writing adjacent PSUM banks. exp / copies / causal
masking / DMA are fused across the pair with 3D access patterns.
"""

import sys

sys.path.insert(0, "/opt/trn_rl_repo")

import numpy as np

import concourse.bass as bass  # noqa: F401  (import registers AP machinery)
from concourse import bacc, bass_utils, mybir
import concourse.tile as tile

B, T, D, H = 4, 1500, 1024, 16
DH = D // H              # 64
NCORES = 8
HPC = H // 2             # 8 heads per core
FPC = HPC * DH           # 512 features per core
NT = (T + 127) // 128    # 12 partition tiles over T (last has 92 rows)
# Column chunk boundaries over T, aligned to the 512-float fp32 PSUM bank
# (a matmul output must not cross a bank boundary).
CS = [0, 512, 1024, T]
NJ = len(CS) - 1
VW = DH + 1              # v columns per head incl. ones column (65)
NV = HPC * VW            # 520

f32 = mybir.dt.float32
f32r = mybir.dt.float32r
EXP = mybir.ActivationFunctionType.Exp
LN = mybir.ActivationFunctionType.Ln
IDENT = mybir.ActivationFunctionType.Identity
GE = mybir.AluOpType.is_ge

_cached_nc = None


def _trows(i):
    return min(128, T - 128 * i)


def _build():
    nc = bacc.Bacc("TRN2", target_bir_lowering=False, debug=False)

    xT = nc.dram_tensor("xT", [D, T], f32r, kind="ExternalInput").ap()
    wqT = nc.dram_tensor("wqT", [D, FPC], f32r, kind="ExternalInput").ap()
    wkT = nc.dram_tensor("wkT", [D, FPC], f32r, kind="ExternalInput").ap()
    wvT = nc.dram_tensor("wvT", [D, FPC], f32r, kind="ExternalInput").ap()
    woT = nc.dram_tensor("woT", [FPC, D], f32r, kind="ExternalInput").ap()
    bqv = nc.dram_tensor("bq", [FPC, 1], f32, kind="ExternalInput").ap()
    bvv = nc.dram_tensor("bv", [1, FPC], f32, kind="ExternalInput").ap()
    vones = nc.dram_tensor("vones", [128, HPC, 1], f32r, kind="ExternalInput").ap()
    qk_out = nc.dram_tensor("qk_out", [HPC, T, T], f32, kind="ExternalOutput").ap()
    outT = nc.dram_tensor("outT", [D, T], f32, kind="ExternalOutput").ap()

    with tile.TileContext(nc) as tc:
        # ---------------- persistent SBUF ----------------
        with tc.tile_pool(name="perm", bufs=1) as perm:
            qT_sb = [perm.tile([128, T], f32r, name=f"qT{m}") for m in range(4)]
            kT_sb = [perm.tile([128, T], f32r, name=f"kT{m}") for m in range(4)]
            v_sb = [perm.tile([128, NV], f32r, name=f"v{i}") for i in range(NT)]
            oT_sb = [perm.tile([128, T], f32r, name=f"oT{m}") for m in range(4)]

            # ---------------- phase 1: QKV projections ----------------
            with tc.tile_pool(name="xw", bufs=1) as xw:
                xT_sb = [xw.tile([128, T], f32r, name=f"x{k}") for k in range(8)]
                for k in range(8):
                    nc.sync.dma_start(xT_sb[k][:], xT[128 * k : 128 * (k + 1), :])

                # q^T = (Wq_p*s) @ x^T (+ bq*s via ACT bias on evacuation)
                # k^T = (Wk_p*s) @ x^T
                with tc.tile_pool(name="wq", bufs=1) as wq, \
                     tc.tile_pool(name="qkps", bufs=2, space="PSUM") as qkps:
                    wq_sb = [wq.tile([128, FPC], f32r, name=f"wq{k}") for k in range(8)]
                    bq_sb = [wq.tile([128, 1], f32, name=f"bq{m}") for m in range(4)]
                    for k in range(8):
                        nc.sync.dma_start(wq_sb[k][:], wqT[128 * k : 128 * (k + 1), :])
                    for m in range(4):
                        nc.sync.dma_start(bq_sb[m][:], bqv[128 * m : 128 * (m + 1), :])
                    for m in range(4):
                        qp = qkps.tile([128, T], f32, tag="qkp")
                        for k in range(8):
                            for j in range(NJ):
                                nc.tensor.matmul(
                                    qp[:, CS[j] : CS[j + 1]],
                                    wq_sb[k][:, 128 * m : 128 * (m + 1)],
                                    xT_sb[k][:, CS[j] : CS[j + 1]],
                                    start=(k == 0), stop=(k == 7),
                                )
                        nc.scalar.activation(qT_sb[m][:], qp[:], IDENT, bias=bq_sb[m][:])

                    wk_sb = [wq.tile([128, FPC], f32r, name=f"wk{k}") for k in range(8)]
                    for k in range(8):
                        nc.sync.dma_start(wk_sb[k][:], wkT[128 * k : 128 * (k + 1), :])
                    for m in range(4):
                        kp = qkps.tile([128, T], f32, tag="qkp")
                        for k in range(8):
                            for j in range(NJ):
                                nc.tensor.matmul(
                                    kp[:, CS[j] : CS[j + 1]],
                                    wk_sb[k][:, 128 * m : 128 * (m + 1)],
                                    xT_sb[k][:, CS[j] : CS[j + 1]],
                                    start=(k == 0), stop=(k == 7),
                                )
                        nc.vector.tensor_copy(kT_sb[m][:], kp[:])

                # v = x @ Wv_p^T + bv; stored interleaved [v_h (64) | 1] * 8.
                # bv is added on evacuation (partition-broadcast once); the
                # ones columns come from a tiny constant DMA.
                with tc.tile_pool(name="wv", bufs=1) as wv, \
                     tc.tile_pool(name="vps", bufs=2, space="PSUM") as vps:
                    wv_sb = [wv.tile([128, FPC], f32r, name=f"wv{k}") for k in range(8)]
                    for k in range(8):
                        nc.sync.dma_start(wv_sb[k][:], wvT[128 * k : 128 * (k + 1), :])
                    bv_row = wv.tile([1, FPC], f32, name="bv_row")
                    nc.sync.dma_start(bv_row[:], bvv[:])
                    bv_bc = wv.tile([128, FPC], f32, name="bv_bc")
                    nc.gpsimd.partition_broadcast(bv_bc[:], bv_row[:])
                    for i in range(NT):
                        rw = _trows(i)
                        nc.sync.dma_start(
                            v_sb[i][0:rw].rearrange("p (h c) -> p h c", c=VW)[:, :, DH : DH + 1],
                            vones[0:rw],
                        )
                        vp = vps.tile([128, FPC], f32, tag="vp")
                        for k in range(8):
                            nc.tensor.matmul(
                                vp[0:rw, :],
                                xT_sb[k][:, 128 * i : 128 * i + rw],
                                wv_sb[k][:],
                                start=(k == 0), stop=(k == 7),
                            )
                        nc.vector.tensor_add(
                            v_sb[i][0:rw].rearrange("p (h c) -> p h c", c=VW)[:, :, 0:DH],
                            vp[0:rw].rearrange("p (h c) -> p h c", c=DH),
                            bv_bc[0:rw].rearrange("p (h c) -> p h c", c=DH),
                        )

            # ---------------- phase 2: attention, head pairs ----------------
            # pair t: head A = 2t (partitions 0:64), head B = 2t+1 (64:128)
            with tc.tile_pool(name="sstage", bufs=3) as sstage, \
                 tc.tile_pool(name="et", bufs=6) as etp, \
                 tc.tile_pool(name="nrm", bufs=3) as nrm, \
                 tc.tile_pool(name="sps", bufs=1, space="PSUM") as sps, \
                 tc.tile_pool(name="stps", bufs=2, space="PSUM") as stps, \
                 tc.tile_pool(name="pops", bufs=1, space="PSUM") as pops:
                for t in range(4):
                    hA = 2 * t
                    qt, kt = qT_sb[t], kT_sb[t]

                    # --- scores S[tq, tk] for the qk output (valid prefix) ---
                    # paired: bank0 = head A, bank1 = head B; st cols
                    # [0,T) = A, [T,2T) = B
                    for i in range(NT):
                        rw = _trows(i)
                        vend = min(128 * i + 128, T)
                        st = sstage.tile([128, 2 * T], f32, tag="sstage")
                        for j in range(NJ):
                            if CS[j] >= vend:
                                break
                            ce = min(CS[j + 1], vend)
                            w = ce - CS[j]
                            sp = sps.tile([128, 1024], f32, tag="sp")
                            nc.tensor.matmul(
                                sp[0:rw, 0 : CS[j + 1] - CS[j]],
                                qt[0:64, 128 * i : 128 * i + rw],
                                kt[0:64, CS[j] : CS[j + 1]],
                                start=True, stop=True, tile_position=(0, 0),
                            )
                            nc.tensor.matmul(
                                sp[0:rw, 512 : 512 + CS[j + 1] - CS[j]],
                                qt[64:128, 128 * i : 128 * i + rw],
                                kt[64:128, CS[j] : CS[j + 1]],
                                start=True, stop=True, tile_position=(64, 0),
                            )
                            src = sp[0:rw].rearrange("p (g c) -> p g c", c=512)[:, :, 0:w]
                            dst = st[0:rw].rearrange("p (g c) -> p g c", c=T)[:, :, CS[j] : ce]
                            nc.vector.tensor_copy(dst, src)
                        # causal -inf fill on the diagonal window [128i, vend)
                        nc.gpsimd.affine_select(
                            st[0:rw].rearrange("p (g c) -> p g c", c=T)[:, :, 128 * i : vend],
                            st[0:rw].rearrange("p (g c) -> p g c", c=T)[:, :, 128 * i : vend],
                            pattern=[[0, 2], [-1, vend - 128 * i]],
                            compare_op=GE, fill=float("-inf"),
                            base=0, channel_multiplier=1,
                        )
                        nc.sync.dma_start(
                            qk_out[hA : hA + 2, 128 * i : 128 * i + rw, 0:vend]
                            .rearrange("h r c -> r h c"),
                            st[0:rw].rearrange("p (g c) -> p g c", c=T)[:, :, 0:vend],
                        )

                    # --- S^T -> exp -> E^T -> po += v_aug.T @ E^T, paired ---
                    for j in range(NJ):
                        wj = CS[j + 1] - CS[j]
                        po = pops.tile([VW, 1024], f32, tag="po")
                        ms = [m for m in range(NT) if 128 * m < CS[j + 1]]
                        for n, m in enumerate(ms):
                            rw = _trows(m)
                            c0 = 128 * m
                            stp = stps.tile([128, 1024], f32, tag="stp")
                            nc.tensor.matmul(
                                stp[0:rw, 0:wj],
                                kt[0:64, c0 : c0 + rw],
                                qt[0:64, CS[j] : CS[j + 1]],
                                start=True, stop=True, tile_position=(0, 0),
                            )
                            nc.tensor.matmul(
                                stp[0:rw, 512 : 512 + wj],
                                kt[64:128, c0 : c0 + rw],
                                qt[64:128, CS[j] : CS[j + 1]],
                                start=True, stop=True, tile_position=(64, 0),
                            )
                            et = etp.tile([128, 1024], f32r, tag="et")
                            cs = max(c0 - CS[j], 0)      # in-tile valid start
                            nc.scalar.activation(
                                et[0:rw].rearrange("p (g c) -> p g c", c=512)[:, :, cs:wj],
                                stp[0:rw].rearrange("p (g c) -> p g c", c=512)[:, :, cs:wj],
                                EXP,
                            )
                            # zero strictly-below-diagonal (and the unwritten
                            # [0, cs) prefix): keep where tq - tk >= 0
                            me = min(c0 + 128, CS[j + 1]) - CS[j]
                            if c0 + 128 > CS[j] and me > 0:
                                nc.gpsimd.affine_select(
                                    et[0:rw].rearrange("p (g c) -> p g c", c=512)[:, :, 0:me],
                                    et[0:rw].rearrange("p (g c) -> p g c", c=512)[:, :, 0:me],
                                    pattern=[[0, 2], [1, me]],
                                    compare_op=GE, fill=0.0,
                                    base=CS[j] - c0, channel_multiplier=-1,
                                )
                            nc.tensor.matmul(
                                po[:, 0:wj],
                                v_sb[m][0:rw, VW * hA : VW * hA + VW],
                                et[0:rw, 0:wj],
                                start=(n == 0), stop=(n == len(ms) - 1),
                            )
                            nc.tensor.matmul(
                                po[:, 512 : 512 + wj],
                                v_sb[m][0:rw, VW * (hA + 1) : VW * (hA + 1) + VW],
                                et[0:rw, 512 : 512 + wj],
                                start=(n == 0), stop=(n == len(ms) - 1),
                            )

                        # --- normalize: out_h^T[:, j] = po[0:64] * (1/po[64]) ---
                        # 1/d on ACT as exp(-ln(d)): Ln and Exp share the
                        # natural_log_exp table set; DVE InstReciprocal is
                        # ~6.4 ns/element and far too slow.
                        lsb = nrm.tile([1, 1024], f32, tag="lsb")
                        rsb = nrm.tile([1, 1024], f32, tag="rsb")
                        pod = po[DH : DH + 1].rearrange("p (g c) -> p g c", c=512)[:, :, 0:wj]
                        lsbv = lsb.rearrange("p (g c) -> p g c", c=512)[:, :, 0:wj]
                        rsbv = rsb.rearrange("p (g c) -> p g c", c=512)[:, :, 0:wj]
                        nc.scalar.activation(lsbv, pod, LN)
                        nc.scalar.activation(rsbv, lsbv, EXP, scale=-1.0)
                        rbc = nrm.tile([DH, 1024], f32, tag="rbc")
                        nc.gpsimd.partition_broadcast(rbc[:], rsb[:])
                        nc.vector.tensor_mul(
                            oT_sb[t][0:DH, CS[j] : CS[j + 1]],
                            po[0:DH, 0:wj], rbc[:, 0:wj],
                        )
                        nc.vector.tensor_mul(
                            oT_sb[t][DH:128, CS[j] : CS[j + 1]],
                            po[0:DH, 512 : 512 + wj], rbc[:, 512 : 512 + wj],
                        )

            # ---------------- phase 3: output projection ----------------
            with tc.tile_pool(name="wo", bufs=1) as wo, \
                 tc.tile_pool(name="ostage", bufs=2) as ostage, \
                 tc.tile_pool(name="ops", bufs=2, space="PSUM") as ops:
                wo_sb = [wo.tile([128, D], f32r, name=f"wo{k}") for k in range(4)]
                for k in range(4):
                    nc.sync.dma_start(wo_sb[k][:], woT[128 * k : 128 * (k + 1), :])
                for n in range(8):
                    pp = ops.tile([128, T], f32, tag="pp")
                    for k in range(4):
                        for j in range(NJ):
                            nc.tensor.matmul(
                                pp[:, CS[j] : CS[j + 1]],
                                wo_sb[k][:, 128 * n : 128 * (n + 1)],
                                oT_sb[k][:, CS[j] : CS[j + 1]],
                                start=(k == 0), stop=(k == 3),
                            )
                    ot = ostage.tile([128, T], f32, tag="ot")
                    nc.scalar.activation(ot[:], pp[:], IDENT)
                    nc.sync.dma_start(outT[128 * n : 128 * (n + 1), :], ot[:])

    nc.compile()
    return nc


def _get_nc():
    global _cached_nc
    if _cached_nc is None:
        _cached_nc = _build()
    return _cached_nc


def kernel(x, mask, Wq, bq, Wk, Wv, bv, Wo, bo, _run_kwargs=None):
    x = np.asarray(x, dtype=np.float32)
    Wq = np.asarray(Wq, dtype=np.float32)
    bq = np.asarray(bq, dtype=np.float32)
    Wk = np.asarray(Wk, dtype=np.float32)
    Wv = np.asarray(Wv, dtype=np.float32)
    bv = np.asarray(bv, dtype=np.float32)
    Wo = np.asarray(Wo, dtype=np.float32)
    bo = np.asarray(bo, dtype=np.float32)

    nc = _get_nc()
    s = float(DH) ** -0.25

    in_maps = []
    for c in range(NCORES):
        b, hg = divmod(c, 2)
        sl = slice(hg * FPC, (hg + 1) * FPC)
        in_maps.append({
            "xT": np.ascontiguousarray(x[b].T),
            "wqT": np.ascontiguousarray((Wq[sl] * s).T),
            "wkT": np.ascontiguousarray((Wk[sl] * s).T),
            "wvT": np.ascontiguousarray(Wv[sl].T),
            "woT": np.ascontiguousarray(Wo[:, sl].T),
            "bq": (bq[sl] * s).reshape(FPC, 1).astype(np.float32),
            "bv": bv[sl].reshape(1, FPC).astype(np.float32),
            "vones": np.ones((128, HPC, 1), np.float32),
        })

    res = bass_utils.run_bass_kernel_spmd(
        nc, in_maps, core_ids=list(range(NCORES)), **(_run_kwargs or {})
    )

    out = np.empty((B, T, D), np.float32)
    qk = np.empty((B, H, T, T), np.float32)
    for b in range(B):
        r0 = res.results[2 * b]
        r1 = res.results[2 * b + 1]
        out[b] = r0["outT"].T + r1["outT"].T + bo
        for hg, r in ((0, r0), (1, r1)):
            for hl in range(HPC):
                h = hg * HPC + hl
                dst = qk[b, h]
                src = r["qk_out"][hl]
                for i in range(NT):
                    ra, rb = 128 * i, 128 * i + _trows(i)
                    vend = min(128 * i + 128, T)
                    dst[ra:rb, :vend] = src[ra:rb, :vend]
                    dst[ra:rb, vend:] = -np.inf
    if _run_kwargs is not None:
        return (out, qk), res
    return out, qk


# revision 28
# speedup vs baseline: 1.3869x; 1.0246x over previous
"""Multi-head attention (whisper-style, returns (out, qk)) on 8 Trainium2 cores.

Sharding: core c -> (batch b = c//2, head-group hg = c%2). Each core computes
8 heads (512 features) of one batch: QKV projections, causal attention scores
(returned as qk), softmax, attention-weighted V, and a partial output
projection. Host sums the two head-group partials per batch and adds bo.

All matmuls run in float32r (TF32-like fast path). Heads are processed in
pairs: the two heads of a qT/kT tile live at partitions 0-63 / 64-127, and
their K=64 score matmuls are packed into the PE array concurrently via
tile_position row groups, writing adjacent PSUM banks. exp / copies / causal
masking / DMA are fused across the pair with 3D access patterns.
"""

import sys

sys.path.insert(0, "/opt/trn_rl_repo")

import numpy as np

import concourse.bass as bass  # noqa: F401  (import registers AP machinery)
from concourse import bacc, bass_utils, mybir
import concourse.tile as tile

B, T, D, H = 4, 1500, 1024, 16
DH = D // H              # 64
NCORES = 8
HPC = H // 2             # 8 heads per core
FPC = HPC * DH           # 512 features per core
NT = (T + 127) // 128    # 12 partition tiles over T (last has 92 rows)
# Column chunk boundaries over T, aligned to the 512-float fp32 PSUM bank
# (a matmul output must not cross a bank boundary).
CS = [0, 512, 1024, T]
NJ = len(CS) - 1
VW = DH + 1              # v columns per head incl. ones column (65)
NV = HPC * VW            # 520

f32 = mybir.dt.float32
f32r = mybir.dt.float32r
EXP = mybir.ActivationFunctionType.Exp
LN = mybir.ActivationFunctionType.Ln
IDENT = mybir.ActivationFunctionType.Identity
GE = mybir.AluOpType.is_ge

_cached_nc = None


def _trows(i):
    return min(128, T - 128 * i)


def _build():
    nc = bacc.Bacc("TRN2", target_bir_lowering=False, debug=False)

    xT = nc.dram_tensor("xT", [D, T], f32r, kind="ExternalInput").ap()
    wqT = nc.dram_tensor("wqT", [D, FPC], f32r, kind="ExternalInput").ap()
    wkT = nc.dram_tensor("wkT", [D, FPC], f32r, kind="ExternalInput").ap()
    wvT = nc.dram_tensor("wvT", [D, FPC], f32r, kind="ExternalInput").ap()
    woT = nc.dram_tensor("woT", [FPC, D], f32r, kind="ExternalInput").ap()
    bqv = nc.dram_tensor("bq", [FPC, 1], f32, kind="ExternalInput").ap()
    bvv = nc.dram_tensor("bv", [1, FPC], f32, kind="ExternalInput").ap()
    vones = nc.dram_tensor("vones", [128, HPC, 1], f32r, kind="ExternalInput").ap()
    qk_out = nc.dram_tensor("qk_out", [HPC, T, T], f32, kind="ExternalOutput").ap()
    outT = nc.dram_tensor("outT", [D, T], f32, kind="ExternalOutput").ap()

    with tile.TileContext(nc) as tc:
        # ---------------- persistent SBUF ----------------
        with tc.tile_pool(name="perm", bufs=1) as perm:
            qT_sb = [perm.tile([128, T], f32r, name=f"qT{m}") for m in range(4)]
            kT_sb = [perm.tile([128, T], f32r, name=f"kT{m}") for m in range(4)]
            v_sb = [perm.tile([128, NV], f32r, name=f"v{i}") for i in range(NT)]
            oT_sb = [perm.tile([128, T], f32r, name=f"oT{m}") for m in range(4)]

            # ---------------- phase 1: QKV projections ----------------
            with tc.tile_pool(name="xw", bufs=1) as xw:
                xT_sb = [xw.tile([128, T], f32r, name=f"x{k}") for k in range(8)]
                for k in range(8):
                    nc.sync.dma_start(xT_sb[k][:], xT[128 * k : 128 * (k + 1), :])

                # q^T = (Wq_p*s) @ x^T (+ bq*s via ACT bias on evacuation)
                # k^T = (Wk_p*s) @ x^T
                with tc.tile_pool(name="wq", bufs=1) as wq, \
                     tc.tile_pool(name="qkps", bufs=2, space="PSUM") as qkps:
                    wq_sb = [wq.tile([128, FPC], f32r, name=f"wq{k}") for k in range(8)]
                    bq_sb = [wq.tile([128, 1], f32, name=f"bq{m}") for m in range(4)]
                    for k in range(8):
                        nc.sync.dma_start(wq_sb[k][:], wqT[128 * k : 128 * (k + 1), :])
                    for m in range(4):
                        nc.sync.dma_start(bq_sb[m][:], bqv[128 * m : 128 * (m + 1), :])
                    for m in range(4):
                        qp = qkps.tile([128, T], f32, tag="qkp")
                        for k in range(8):
                            for j in range(NJ):
                                nc.tensor.matmul(
                                    qp[:, CS[j] : CS[j + 1]],
                                    wq_sb[k][:, 128 * m : 128 * (m + 1)],
                                    xT_sb[k][:, CS[j] : CS[j + 1]],
                                    start=(k == 0), stop=(k == 7),
                                )
                        nc.scalar.activation(qT_sb[m][:], qp[:], IDENT, bias=bq_sb[m][:])

                    wk_sb = [wq.tile([128, FPC], f32r, name=f"wk{k}") for k in range(8)]
                    for k in range(8):
                        nc.sync.dma_start(wk_sb[k][:], wkT[128 * k : 128 * (k + 1), :])
                    for m in range(4):
                        kp = qkps.tile([128, T], f32, tag="qkp")
                        for k in range(8):
                            for j in range(NJ):
                                nc.tensor.matmul(
                                    kp[:, CS[j] : CS[j + 1]],
                                    wk_sb[k][:, 128 * m : 128 * (m + 1)],
                                    xT_sb[k][:, CS[j] : CS[j + 1]],
                                    start=(k == 0), stop=(k == 7),
                                )
                        nc.vector.tensor_copy(kT_sb[m][:], kp[:])

                # v = x @ Wv_p^T + bv; stored interleaved [v_h (64) | 1] * 8.
                # bv is added on evacuation (partition-broadcast once); the
                # ones columns come from a tiny constant DMA.
                with tc.tile_pool(name="wv", bufs=1) as wv, \
                     tc.tile_pool(name="vps", bufs=2, space="PSUM") as vps:
                    wv_sb = [wv.tile([128, FPC], f32r, name=f"wv{k}") for k in range(8)]
                    for k in range(8):
                        nc.sync.dma_start(wv_sb[k][:], wvT[128 * k : 128 * (k + 1), :])
                    bv_row = wv.tile([1, FPC], f32, name="bv_row")
                    nc.sync.dma_start(bv_row[:], bvv[:])
                    bv_bc = wv.tile([128, FPC], f32, name="bv_bc")
                    nc.gpsimd.partition_broadcast(bv_bc[:], bv_row[:])
                    for i in range(NT):
                        rw = _trows(i)
                        nc.sync.dma_start(
                            v_sb[i][0:rw].rearrange("p (h c) -> p h c", c=VW)[:, :, DH : DH + 1],
                            vones[0:rw],
                        )
                        vp = vps.tile([128, FPC], f32, tag="vp")
                        for k in range(8):
                            nc.tensor.matmul(
                                vp[0:rw, :],
                                xT_sb[k][:, 128 * i : 128 * i + rw],
                                wv_sb[k][:],
                                start=(k == 0), stop=(k == 7),
                            )
                        nc.vector.tensor_add(
                            v_sb[i][0:rw].rearrange("p (h c) -> p h c", c=VW)[:, :, 0:DH],
                            vp[0:rw].rearrange("p (h c) -> p h c", c=DH),
                            bv_bc[0:rw].rearrange("p (h c) -> p h c", c=DH),
                        )

            # ---------------- phase 2: attention, head pairs ----------------
            # pair t: head A = 2t (partitions 0:64), head B = 2t+1 (64:128)
            with tc.tile_pool(name="sstage", bufs=3) as sstage, \
                 tc.tile_pool(name="et", bufs=6) as etp, \
                 tc.tile_pool(name="nrm", bufs=3) as nrm, \
                 tc.tile_pool(name="sps", bufs=1, space="PSUM") as sps, \
                 tc.tile_pool(name="stps", bufs=2, space="PSUM") as stps, \
                 tc.tile_pool(name="pops", bufs=1, space="PSUM") as pops:
                for t in range(4):
                    hA = 2 * t
                    qt, kt = qT_sb[t], kT_sb[t]

                    # --- scores S[tq, tk] for the qk output (valid prefix) ---
                    # paired: bank0 = head A, bank1 = head B; st cols
                    # [0,T) = A, [T,2T) = B
                    for i in range(NT):
                        rw = _trows(i)
                        vend = min(128 * i + 128, T)
                        st = sstage.tile([128, 2 * T], f32, tag="sstage")
                        for j in range(NJ):
                            if CS[j] >= vend:
                                break
                            ce = min(CS[j + 1], vend)
                            w = ce - CS[j]
                            sp = sps.tile([128, 1024], f32, tag="sp")
                            nc.tensor.matmul(
                                sp[0:rw, 0 : CS[j + 1] - CS[j]],
                                qt[0:64, 128 * i : 128 * i + rw],
                                kt[0:64, CS[j] : CS[j + 1]],
                                start=True, stop=True, tile_position=(0, 0),
                            )
                            nc.tensor.matmul(
                                sp[0:rw, 512 : 512 + CS[j + 1] - CS[j]],
                                qt[64:128, 128 * i : 128 * i + rw],
                                kt[64:128, CS[j] : CS[j + 1]],
                                start=True, stop=True, tile_position=(64, 0),
                            )
                            src = sp[0:rw].rearrange("p (g c) -> p g c", c=512)[:, :, 0:w]
                            dst = st[0:rw].rearrange("p (g c) -> p g c", c=T)[:, :, CS[j] : ce]
                            if (i * NJ + j) % 4 == 0:
                                nc.scalar.activation(dst, src, IDENT)
                            else:
                                nc.vector.tensor_copy(dst, src)
                        # causal -inf fill on the diagonal window [128i, vend)
                        nc.gpsimd.affine_select(
                            st[0:rw].rearrange("p (g c) -> p g c", c=T)[:, :, 128 * i : vend],
                            st[0:rw].rearrange("p (g c) -> p g c", c=T)[:, :, 128 * i : vend],
                            pattern=[[0, 2], [-1, vend - 128 * i]],
                            compare_op=GE, fill=float("-inf"),
                            base=0, channel_multiplier=1,
                        )
                        nc.sync.dma_start(
                            qk_out[hA : hA + 2, 128 * i : 128 * i + rw, 0:vend]
                            .rearrange("h r c -> r h c"),
                            st[0:rw].rearrange("p (g c) -> p g c", c=T)[:, :, 0:vend],
                        )

                    # --- S^T -> exp -> E^T -> po += v_aug.T @ E^T, paired ---
                    for j in range(NJ):
                        wj = CS[j + 1] - CS[j]
                        po = pops.tile([VW, 1024], f32, tag="po")
                        ms = [m for m in range(NT) if 128 * m < CS[j + 1]]
                        for n, m in enumerate(ms):
                            rw = _trows(m)
                            c0 = 128 * m
                            stp = stps.tile([128, 1024], f32, tag="stp")
                            nc.tensor.matmul(
                                stp[0:rw, 0:wj],
                                kt[0:64, c0 : c0 + rw],
                                qt[0:64, CS[j] : CS[j + 1]],
                                start=True, stop=True, tile_position=(0, 0),
                            )
                            nc.tensor.matmul(
                                stp[0:rw, 512 : 512 + wj],
                                kt[64:128, c0 : c0 + rw],
                                qt[64:128, CS[j] : CS[j + 1]],
                                start=True, stop=True, tile_position=(64, 0),
                            )
                            et = etp.tile([128, 1024], f32r, tag="et")
                            cs = max(c0 - CS[j], 0)      # in-tile valid start
                            nc.scalar.activation(
                                et[0:rw].rearrange("p (g c) -> p g c", c=512)[:, :, cs:wj],
                                stp[0:rw].rearrange("p (g c) -> p g c", c=512)[:, :, cs:wj],
                                EXP,
                            )
                            # zero strictly-below-diagonal (and the unwritten
                            # [0, cs) prefix): keep where tq - tk >= 0
                            me = min(c0 + 128, CS[j + 1]) - CS[j]
                            if c0 + 128 > CS[j] and me > 0:
                                nc.gpsimd.affine_select(
                                    et[0:rw].rearrange("p (g c) -> p g c", c=512)[:, :, 0:me],
                                    et[0:rw].rearrange("p (g c) -> p g c", c=512)[:, :, 0:me],
                                    pattern=[[0, 2], [1, me]],
                                    compare_op=GE, fill=0.0,
                                    base=CS[j] - c0, channel_multiplier=-1,
                                )
                            nc.tensor.matmul(
                                po[:, 0:wj],
                                v_sb[m][0:rw, VW * hA : VW * hA + VW],
                                et[0:rw, 0:wj],
                                start=(n == 0), stop=(n == len(ms) - 1),
                            )
                            nc.tensor.matmul(
                                po[:, 512 : 512 + wj],
                                v_sb[m][0:rw, VW * (hA + 1) : VW * (hA + 1) + VW],
                                et[0:rw, 512 : 512 + wj],
                                start=(n == 0), stop=(n == len(ms) - 1),
                            )

                        # --- normalize: out_h^T[:, j] = po[0:64] * (1/po[64]) ---
                        # d >= min exp > 0 and bounded, so the fast
                        # bit-trick reciprocal (~18 correct bits) is safe;
                        # DVE InstReciprocal is ~6.4 ns/element, way too slow.
                        dsb = nrm.tile([1, 1024], f32, tag="dsb")
                        rsb = nrm.tile([1, 1024], f32, tag="rsb")
                        pod = po[DH : DH + 1].rearrange("p (g c) -> p g c", c=512)[:, :, 0:wj]
                        dsbv = dsb.rearrange("p (g c) -> p g c", c=512)[:, :, 0:wj]
                        rsbv = rsb.rearrange("p (g c) -> p g c", c=512)[:, :, 0:wj]
                        nc.scalar.activation(dsbv, pod, IDENT)
                        nc.vector.reciprocal_approx_fast(out=rsbv, in_=dsbv)
                        rbc = nrm.tile([DH, 1024], f32, tag="rbc")
                        nc.gpsimd.partition_broadcast(rbc[:], rsb[:])
                        nc.vector.tensor_mul(
                            oT_sb[t][0:DH, CS[j] : CS[j + 1]],
                            po[0:DH, 0:wj], rbc[:, 0:wj],
                        )
                        nc.vector.tensor_mul(
                            oT_sb[t][DH:128, CS[j] : CS[j + 1]],
                            po[0:DH, 512 : 512 + wj], rbc[:, 512 : 512 + wj],
                        )

            # ---------------- phase 3: output projection ----------------
            with tc.tile_pool(name="wo", bufs=1) as wo, \
                 tc.tile_pool(name="ostage", bufs=2) as ostage, \
                 tc.tile_pool(name="ops", bufs=2, space="PSUM") as ops:
                wo_sb = [wo.tile([128, D], f32r, name=f"wo{k}") for k in range(4)]
                for k in range(4):
                    nc.sync.dma_start(wo_sb[k][:], woT[128 * k : 128 * (k + 1), :])
                for n in range(8):
                    pp = ops.tile([128, T], f32, tag="pp")
                    for k in range(4):
                        for j in range(NJ):
                            nc.tensor.matmul(
                                pp[:, CS[j] : CS[j + 1]],
                                wo_sb[k][:, 128 * n : 128 * (n + 1)],
                                oT_sb[k][:, CS[j] : CS[j + 1]],
                                start=(k == 0), stop=(k == 3),
                            )
                    ot = ostage.tile([128, T], f32, tag="ot")
                    nc.scalar.activation(ot[:], pp[:], IDENT)
                    nc.sync.dma_start(outT[128 * n : 128 * (n + 1), :], ot[:])

    nc.compile()
    return nc


def _get_nc():
    global _cached_nc
    if _cached_nc is None:
        _cached_nc = _build()
    return _cached_nc


def kernel(x, mask, Wq, bq, Wk, Wv, bv, Wo, bo, _run_kwargs=None):
    x = np.asarray(x, dtype=np.float32)
    Wq = np.asarray(Wq, dtype=np.float32)
    bq = np.asarray(bq, dtype=np.float32)
    Wk = np.asarray(Wk, dtype=np.float32)
    Wv = np.asarray(Wv, dtype=np.float32)
    bv = np.asarray(bv, dtype=np.float32)
    Wo = np.asarray(Wo, dtype=np.float32)
    bo = np.asarray(bo, dtype=np.float32)

    nc = _get_nc()
    s = float(DH) ** -0.25

    in_maps = []
    for c in range(NCORES):
        b, hg = divmod(c, 2)
        sl = slice(hg * FPC, (hg + 1) * FPC)
        in_maps.append({
            "xT": np.ascontiguousarray(x[b].T),
            "wqT": np.ascontiguousarray((Wq[sl] * s).T),
            "wkT": np.ascontiguousarray((Wk[sl] * s).T),
            "wvT": np.ascontiguousarray(Wv[sl].T),
            "woT": np.ascontiguousarray(Wo[:, sl].T),
            "bq": (bq[sl] * s).reshape(FPC, 1).astype(np.float32),
            "bv": bv[sl].reshape(1, FPC).astype(np.float32),
            "vones": np.ones((128, HPC, 1), np.float32),
        })

    res = bass_utils.run_bass_kernel_spmd(
        nc, in_maps, core_ids=list(range(NCORES)), **(_run_kwargs or {})
    )

    out = np.empty((B, T, D), np.float32)
    qk = np.empty((B, H, T, T), np.float32)
    for b in range(B):
        r0 = res.results[2 * b]
        r1 = res.results[2 * b + 1]
        out[b] = r0["outT"].T + r1["outT"].T + bo
        for hg, r in ((0, r0), (1, r1)):
            for hl in range(HPC):
                h = hg * HPC + hl
                dst = qk[b, h]
                src = r["qk_out"][hl]
                for i in range(NT):
                    ra, rb = 128 * i, 128 * i + _trows(i)
                    vend = min(128 * i + 128, T)
                    dst[ra:rb, :vend] = src[ra:rb, :vend]
                    dst[ra:rb, vend:] = -np.inf
    if _run_kwargs is not None:
        return (out, qk), res
    return out, qk
